# revision 8
# baseline (speedup 1.0000x reference)
"""Trainium2 Bass kernel for nn_MixA_Module (channel avg/max pool -> sigmoid ->
softmax over spatial -> broadcast multiply).

Reference (per (b,t) frame, x slice is (C=64, S=4096)):
    avg = sigmoid(mean_c x); mx = sigmoid(max_c x); fusion = avg + mx + skin
    attn = softmax_S(fusion); out = x * attn

Sharding: data-parallel over (B, T). 8 cores; core k handles b = k//2 and a
16-long t range, processed as 8 t-pairs so tiles have 128 partitions.

Per t-pair on chip (natural layout: partition = (t2, c), free = s = w*64+h):
  1. DMA x pair tile (128, 4096), 16 KiB/partition contiguous runs.
  2. PE transpose-matmuls per 128-col chunk with rhs = [I128 | block-ones(2)]:
     psum chunk (128, 130) = [x_chunk^T | per-s channel sums for t2=0,1].
  3. DVE reduce_max over the transposed part -> max over c, packed (128, 64)
     as (s_local, t2*32 + chunk).  ACT tanh on the sums cols gives
     sigmoid(mean) via sigmoid(v) = 0.5*tanh(v/2) + 0.5 (tanh shares the
     "exp_and_others" ACT table set with exp - no table reloads).
  4. fusion exp: E = exp(0.5*(tanh_avg + tanh_max + 2*skin + 2)) on ACT with
     accum_out giving per-partition partial sums; tiny PE matmul + reciprocal
     give 1/S per t2.  E (unnormalized) and S are returned to host, which
     computes attn = E/S (host divide matches jax softmax's fp32 divide).
  5. E is PE-transposed and SBUF->SBUF DMA-flattened to E2 (2, 4096); a PE
     matmul with lhsT = blockmask * (1/S) broadcasts attn = E/S to all 128
     partitions in PSUM; DVE tensor_mul produces out = x * attn.
  6. DMA out tile back in x's layout.
"""

import numpy as np

B, C, T, W, H = 4, 64, 32, 64, 64
S = W * H            # 4096 spatial positions per frame
NCORES = 8
TPC = T * B // NCORES  # 16 t-slices per core (b fixed per core)
PAIRS = TPC // 2       # 8 t-pairs per core
NCHUNK = S // 128      # 32 transpose chunks per pair
GRP = 4                # chunks per psum group
NGRP = NCHUNK // GRP   # 8 groups per pair

# Transpose-mode matmuls require a permutation rhs (HW restriction), so the
# [I | ones] combo matmul and the broadcast matmul run as regular fp32 matmuls.


def build_program():
    """Build the single-core SPMD Bass program (same program, per-core data)."""
    from contextlib import ExitStack

    import concourse.bacc as bacc
    import concourse.bass as bass
    import concourse.tile as tile
    from concourse import mybir

    f32 = mybir.dt.float32
    AF = mybir.ActivationFunctionType

    nc = bacc.Bacc(
        "TRN2",
        target_bir_lowering=False,
        debug=False,
        enable_asserts=False,
        num_devices=NCORES,
    )

    xs = nc.dram_tensor("xs", [C, TPC, S], f32, kind="ExternalInput").ap()
    skinp = nc.dram_tensor("skinp", [128, PAIRS * 64], f32, kind="ExternalInput").ap()
    ident = nc.dram_tensor("ident", [128, 130], f32, kind="ExternalInput").ap()
    bmask2 = nc.dram_tensor("bmask2", [2, 128], f32, kind="ExternalInput").ap()
    ones128 = nc.dram_tensor("ones128", [128, 1], f32, kind="ExternalInput").ap()
    outx = nc.dram_tensor("outx", [C, TPC, S], f32, kind="ExternalOutput").ap()
    epk = nc.dram_tensor("epk", [128, PAIRS * 64], f32, kind="ExternalOutput").ap()
    sall = nc.dram_tensor("sall", [2, PAIRS], f32, kind="ExternalOutput").ap()

    with tile.TileContext(nc) as tc, ExitStack() as ctx:
        singles = ctx.enter_context(tc.tile_pool(name="singles", bufs=1))
        xpool = ctx.enter_context(tc.tile_pool(name="xin", bufs=3))
        opool = ctx.enter_context(tc.tile_pool(name="oout", bufs=3))
        packs = ctx.enter_context(tc.tile_pool(name="packs", bufs=2))
        e2pool = ctx.enter_context(tc.tile_pool(name="e2", bufs=2))
        psT = ctx.enter_context(tc.tile_pool(name="psT", bufs=2, space="PSUM"))
        psB = ctx.enter_context(tc.tile_pool(name="psB", bufs=2, space="PSUM"))
        psM = ctx.enter_context(tc.tile_pool(name="psM", bufs=2, space="PSUM"))

        ident_sb = singles.tile([128, 130], f32)
        nc.sync.dma_start(ident_sb[:, :], ident)
        bmask_sb = singles.tile([2, 128], f32)
        nc.sync.dma_start(bmask_sb[:, :], bmask2)
        ones_sb = singles.tile([128, 1], f32)
        nc.sync.dma_start(ones_sb[:, :], ones128)
        skin_sb = singles.tile([128, PAIRS * 64], f32)
        nc.sync.dma_start(skin_sb[:, :], skinp)
        epk_sb = singles.tile([128, PAIRS * 64], f32)
        sall_sb = singles.tile([2, PAIRS], f32)

        for j in range(PAIRS):
            # --- load x pair: DRAM walk order (t, c, s) -> partitions (t2, c)
            xt = xpool.tile([128, S], f32)
            nc.sync.dma_start(
                xt[:, :], xs[:, 2 * j : 2 * j + 2, :].rearrange("c t s -> t c s")
            )

            mxp = packs.tile([128, 64], f32)   # max over c, (s_local, t2*32+chunk)
            tavg = packs.tile([128, 64], f32)  # tanh(mean/2), same packing

            for g in range(NGRP):
                tg = psT.tile([128, GRP * 256], f32)
                for ci in range(GRP):
                    i = GRP * g + ci
                    # out chunk = [x_chunk^T | channel sums t2=0 | t2=1]
                    nc.tensor.matmul(
                        tg[:, 256 * ci : 256 * ci + 130],
                        xt[:, 128 * i : 128 * (i + 1)],
                        ident_sb[:, :],
                    )
                tg3 = tg[:, :].rearrange("p (ch q) -> p ch q", ch=GRP)
                # max over c: in (p, ch, t2, c) -> out (p, ch, t2)
                in_ap = tg3[:, :, 0:128].rearrange("p ch (t2 c) -> p ch t2 c", t2=2)
                out_ap = (
                    mxp[:, :]
                    .rearrange("p (t2 gg) -> p t2 gg", t2=2)[:, :, GRP * g : GRP * (g + 1)]
                    .rearrange("p t2 ch -> p ch t2")
                )
                nc.vector.reduce_max(out=out_ap, in_=in_ap, axis=mybir.AxisListType.X)
                tavg_ap = (
                    tavg[:, :]
                    .rearrange("p (t2 gg) -> p t2 gg", t2=2)[:, :, GRP * g : GRP * (g + 1)]
                    .rearrange("p t2 ch -> p ch t2")
                )
                # sigmoid(sum/64) = 0.5 + 0.5*tanh(sum/128)
                nc.scalar.activation(
                    out=tavg_ap, in_=tg3[:, :, 128:130], func=AF.Tanh,
                    scale=1.0 / 128.0,
                )

            # --- fusion: E = exp(0.5*(tanh_avg + tanh_max + 2*skin + 2))
            tmx = packs.tile([128, 64], f32)
            nc.scalar.activation(out=tmx[:, :], in_=mxp[:, :], func=AF.Tanh, scale=0.5)
            q = packs.tile([128, 64], f32)
            nc.vector.tensor_add(q[:, :], tavg[:, :], tmx[:, :])
            q2 = packs.tile([128, 64], f32)
            nc.vector.tensor_add(q2[:, :], q[:, :], skin_sb[:, 64 * j : 64 * (j + 1)])

            e_sl = epk_sb[:, 64 * j : 64 * (j + 1)]
            epart = packs.tile([128, 2], f32)
            nc.scalar.activation(
                out=e_sl[:, 0:32], in_=q2[:, 0:32], func=AF.Exp, scale=0.5,
                accum_out=epart[:, 0:1],
            )
            nc.scalar.activation(
                out=e_sl[:, 32:64], in_=q2[:, 32:64], func=AF.Exp, scale=0.5,
                accum_out=epart[:, 1:2],
            )

            # --- softmax denominators S[t2] and 1/S
            ssum = psM.tile([2, 1], f32, tag="m")
            nc.tensor.matmul(ssum[:, :], epart[:, :], ones_sb[:, :])
            nc.scalar.copy(sall_sb[:, j : j + 1], ssum[:, :])
            rs = packs.tile([2, 1], f32)
            nc.vector.reciprocal(rs[:, :], ssum[:, :])
            lhsw = packs.tile([2, 128], f32)
            nc.vector.tensor_scalar_mul(lhsw[:, :], bmask_sb[:, :], rs[:, :])

            # --- E2 (2, 4096): transpose packed E then flatten partitions
            etp = psM.tile([64, 128], f32, tag="m")
            nc.tensor.transpose(etp[:, :], e_sl, ident_sb[:, 0:128])
            ets = packs.tile([64, 128], f32)
            nc.scalar.copy(ets[:, :], etp[:, :])
            e2 = e2pool.tile([2, S], f32)
            nc.gpsimd.dma_start(e2[0:1, :], ets[0:32, :])
            nc.gpsimd.dma_start(e2[1:2, :], ets[32:64, :])

            # --- broadcast attn across partitions and multiply
            ot = opool.tile([128, S], f32)
            for m in range(8):
                bp = psB.tile([128, 512], f32)
                nc.tensor.matmul(
                    bp[:, :], lhsw[:, :], e2[:, 512 * m : 512 * (m + 1)],
                )
                nc.vector.tensor_mul(
                    ot[:, 512 * m : 512 * (m + 1)],
                    xt[:, 512 * m : 512 * (m + 1)],
                    bp[:, :],
                )
            nc.sync.dma_start(
                outx[:, 2 * j : 2 * j + 2, :].rearrange("c t s -> t c s"), ot[:, :]
            )

        nc.sync.dma_start(epk, epk_sb[:, :])
        nc.sync.dma_start(sall, sall_sb[:, :])

    nc.compile()
    return nc


def make_consts():
    ident = np.zeros((128, 130), dtype=np.float32)
    ident[:128, :128] = np.eye(128, dtype=np.float32)
    ident[0:64, 128] = 1.0
    ident[64:128, 129] = 1.0
    bmask2 = np.zeros((2, 128), dtype=np.float32)
    bmask2[0, 0:64] = 1.0
    bmask2[1, 64:128] = 1.0
    ones128 = np.ones((128, 1), dtype=np.float32)
    return ident, bmask2, ones128


def core_inputs(x, skin, k):
    """Per-core input dict for core k (host-side shard + pack)."""
    b = k // (NCORES // B)
    tbase = TPC * (k % (NCORES // B))
    xs = np.ascontiguousarray(
        np.asarray(x[b, :, tbase : tbase + TPC], dtype=np.float32).reshape(C, TPC, S)
    )
    sk = np.asarray(skin[b, tbase : tbase + TPC], dtype=np.float32).reshape(TPC, S)
    skh = 2.0 * sk + 2.0
    # skinp[p, j*64 + t2*32 + g] = skh[2j+t2, g*128+p]
    skinp = np.ascontiguousarray(
        skh.reshape(PAIRS, 2, NCHUNK, 128).transpose(3, 0, 1, 2).reshape(128, PAIRS * 64)
    )
    ident, bmask2, ones128 = make_consts()
    return {
        "xs": xs, "skinp": skinp, "ident": ident, "bmask2": bmask2,
        "ones128": ones128,
    }


def assemble(results):
    """Combine per-core outputs into full (output, attn)."""
    output = np.empty((B, C, T, W, H), dtype=np.float32)
    attn = np.empty((B, T, W, H), dtype=np.float32)
    for k in range(NCORES):
        b = k // (NCORES // B)
        tbase = TPC * (k % (NCORES // B))
        r = results[k]
        output[b, :, tbase : tbase + TPC] = r["outx"].reshape(C, TPC, W, H)
        # epk (128, PAIRS*64) -> [p, j, t2, g] -> (t, s)
        e = (
            r["epk"].reshape(128, PAIRS, 2, NCHUNK).transpose(1, 2, 3, 0)
            .reshape(TPC, S)
        )
        s = r["sall"].T.reshape(TPC, 1)  # S[t2, j] -> per-t vector
        attn[b, tbase : tbase + TPC] = (e / s).reshape(TPC, W, H)
    return output, attn


_NC_CACHE = {}


def run(x, skin, trace=False, **spmd_kwargs):
    from concourse.bass_utils import run_bass_kernel_spmd

    if "nc" not in _NC_CACHE:
        _NC_CACHE["nc"] = build_program()
    nc = _NC_CACHE["nc"]
    in_maps = [core_inputs(x, skin, k) for k in range(NCORES)]
    res = run_bass_kernel_spmd(
        nc, in_maps, core_ids=list(range(NCORES)), trace=trace, **spmd_kwargs
    )
    return assemble(res.results), res


def kernel(x, skin):
    out, _ = run(x, skin)
    return out


# revision 16
# speedup vs baseline: 5.6765x; 5.6765x over previous
"""Trainium2 Bass kernel for nn_MixA_Module (channel avg/max pool -> sigmoid ->
softmax over spatial -> broadcast multiply).

Reference (per (b,t) frame, x slice is (C=64, S=4096)):
    avg = sigmoid(mean_c x); mx = sigmoid(max_c x); fusion = avg + mx + skin
    attn = softmax_S(fusion); out = x * attn

Sharding: data-parallel over (B, T). 8 cores; core k handles b = k//2 and a
16-long t range, processed as 8 t-pairs so tiles have 128 partitions.

Per t-pair on chip (natural layout: partition = (t2, c), free = s = w*64+h):
  1. DMA x pair tile (128, 4096), 16 KiB/partition contiguous runs.
  2. PE transpose-matmuls per 128-col chunk with rhs = [I128 | block-ones(2)]:
     psum chunk (128, 130) = [x_chunk^T | per-s channel sums for t2=0,1].
  3. DVE reduce_max over the transposed part -> max over c, packed (128, 64)
     as (s_local, t2*32 + chunk).  ACT tanh on the sums cols gives
     sigmoid(mean) via sigmoid(v) = 0.5*tanh(v/2) + 0.5 (tanh shares the
     "exp_and_others" ACT table set with exp - no table reloads).
  4. fusion exp: E = exp(0.5*(tanh_avg + tanh_max + 2*skin + 2)) on ACT with
     accum_out giving per-partition partial sums; tiny PE matmul + reciprocal
     give 1/S per t2.  E (unnormalized) and S are returned to host, which
     computes attn = E/S (host divide matches jax softmax's fp32 divide).
  5. E is PE-transposed and SBUF->SBUF DMA-flattened to E2 (2, 4096); a PE
     matmul with lhsT = blockmask * (1/S) broadcasts attn = E/S to all 128
     partitions in PSUM; DVE tensor_mul produces out = x * attn.
  6. DMA out tile back in x's layout.
"""

import numpy as np

B, C, T, W, H = 4, 64, 32, 64, 64
S = W * H            # 4096 spatial positions per frame
NCORES = 8
TPC = T * B // NCORES  # 16 t-slices per core (b fixed per core)
PAIRS = TPC // 2       # 8 t-pairs per core
NCHUNK = S // 128      # 32 transpose chunks per pair
GRP = 4                # chunks per psum group
NGRP = NCHUNK // GRP   # 8 groups per pair

# Transpose-mode matmuls require a permutation rhs (HW restriction), so the
# [I | ones] combo matmul and the broadcast matmul run as regular fp32 matmuls.


def build_program(loop_n=None):
    """Build the single-core SPMD Bass program (same program, per-core data).

    loop_n: if set, wrap the whole body in a For_i repeat loop (benchmarking
    only — outputs are identical every iteration)."""
    from contextlib import ExitStack, nullcontext

    import concourse.bacc as bacc
    import concourse.bass as bass
    import concourse.tile as tile
    from concourse import mybir

    f32 = mybir.dt.float32
    AF = mybir.ActivationFunctionType

    nc = bacc.Bacc(
        "TRN2",
        target_bir_lowering=False,
        debug=False,
        enable_asserts=False,
        num_devices=NCORES,
    )

    xs = nc.dram_tensor("xs", [C, TPC, S], f32, kind="ExternalInput").ap()
    skinp = nc.dram_tensor("skinp", [128, PAIRS * 64], f32, kind="ExternalInput").ap()
    ident = nc.dram_tensor("ident", [128, 130], f32, kind="ExternalInput").ap()
    bmask4 = nc.dram_tensor("bmask4", [98, 128], f32, kind="ExternalInput").ap()
    ones128 = nc.dram_tensor("ones128", [128, 1], f32, kind="ExternalInput").ap()
    outx = nc.dram_tensor("outx", [C, TPC, S], f32, kind="ExternalOutput").ap()
    epk = nc.dram_tensor("epk", [128, PAIRS * 64], f32, kind="ExternalOutput").ap()
    sall = nc.dram_tensor("sall", [2, PAIRS], f32, kind="ExternalOutput").ap()

    with tile.TileContext(nc) as tc, ExitStack() as ctx:
        singles = ctx.enter_context(tc.tile_pool(name="singles", bufs=1))
        xpool = ctx.enter_context(tc.tile_pool(name="xin", bufs=3))
        opool = ctx.enter_context(tc.tile_pool(name="oout", bufs=3))
        packs = ctx.enter_context(tc.tile_pool(name="packs", bufs=2))
        e2pool = ctx.enter_context(tc.tile_pool(name="e2", bufs=2))
        psT = ctx.enter_context(tc.tile_pool(name="psT", bufs=2, space="PSUM"))
        psB = ctx.enter_context(tc.tile_pool(name="psB", bufs=3, space="PSUM"))
        psM = ctx.enter_context(tc.tile_pool(name="psM", bufs=1, space="PSUM"))

        ident_sb = singles.tile([128, 130], f32)
        nc.sync.dma_start(ident_sb[:, :], ident)
        bmask_sb = singles.tile([98, 128], f32)
        nc.sync.dma_start(bmask_sb[:, :], bmask4)
        ones_sb = singles.tile([128, 1], f32)
        nc.sync.dma_start(ones_sb[:, :], ones128)
        skin_sb = singles.tile([128, PAIRS * 64], f32)
        nc.sync.dma_start(skin_sb[:, :], skinp)
        epk_sb = singles.tile([128, PAIRS * 64], f32)
        sall_sb = singles.tile([2, PAIRS], f32)

        loop_cm = tc.For_i(0, loop_n, 1) if loop_n else nullcontext()
        with loop_cm:
            _kernel_body(
                nc, tc, mybir, xs, outx, epk, sall, ident_sb, bmask_sb, ones_sb,
                skin_sb, epk_sb, sall_sb, xpool, opool, packs, e2pool, psT, psB,
                psM, f32, AF,
            )

    nc.compile()
    return nc


def _kernel_body(
    nc, tc, mybir, xs, outx, epk, sall, ident_sb, bmask_sb, ones_sb, skin_sb,
    epk_sb, sall_sb, xpool, opool, packs, e2pool, psT, psB, psM, f32, AF,
):
    if True:
        for j in range(PAIRS):
            # --- load x pair: DRAM walk order (t, c, s) -> partitions (t2, c)
            xt = xpool.tile([128, S], f32)
            nc.sync.dma_start(
                xt[:, :], xs[:, 2 * j : 2 * j + 2, :].rearrange("c t s -> t c s")
            )

            mxp = packs.tile([128, 64], f32)   # max over c, (s_local, t2*32+chunk)
            tavg = packs.tile([128, 64], f32)  # tanh(mean/2), same packing

            for g in range(NGRP):
                tg = psT.tile([128, GRP * 256], f32)
                for ci in range(GRP):
                    i = GRP * g + ci
                    # out chunk = [x_chunk^T | channel sums t2=0 | t2=1]
                    nc.tensor.matmul(
                        tg[:, 256 * ci : 256 * ci + 130],
                        xt[:, 128 * i : 128 * (i + 1)],
                        ident_sb[:, :],
                    )
                tg3 = tg[:, :].rearrange("p (ch q) -> p ch q", ch=GRP)
                # max over c: in (p, ch, t2, c) -> out (p, ch, t2)
                in_ap = tg3[:, :, 0:128].rearrange("p ch (t2 c) -> p ch t2 c", t2=2)
                out_ap = (
                    mxp[:, :]
                    .rearrange("p (t2 gg) -> p t2 gg", t2=2)[:, :, GRP * g : GRP * (g + 1)]
                    .rearrange("p t2 ch -> p ch t2")
                )
                nc.vector.reduce_max(out=out_ap, in_=in_ap, axis=mybir.AxisListType.X)
                tavg_ap = (
                    tavg[:, :]
                    .rearrange("p (t2 gg) -> p t2 gg", t2=2)[:, :, GRP * g : GRP * (g + 1)]
                    .rearrange("p t2 ch -> p ch t2")
                )
                # sigmoid(sum/64) = 0.5 + 0.5*tanh(sum/128)
                nc.scalar.activation(
                    out=tavg_ap, in_=tg3[:, :, 128:130], func=AF.Tanh,
                    scale=1.0 / 128.0,
                )

            # --- fusion: E = exp(0.5*(tanh_avg + tanh_max + 2*skin + 2))
            tmx = packs.tile([128, 64], f32)
            nc.scalar.activation(out=tmx[:, :], in_=mxp[:, :], func=AF.Tanh, scale=0.5)
            q = packs.tile([128, 64], f32)
            nc.vector.tensor_add(q[:, :], tavg[:, :], tmx[:, :])
            q2 = packs.tile([128, 64], f32)
            nc.vector.tensor_add(q2[:, :], q[:, :], skin_sb[:, 64 * j : 64 * (j + 1)])

            e_sl = epk_sb[:, 64 * j : 64 * (j + 1)]
            epart = packs.tile([128, 2], f32)
            nc.scalar.activation(
                out=e_sl[:, 0:32], in_=q2[:, 0:32], func=AF.Exp, scale=0.5,
                accum_out=epart[:, 0:1],
            )
            nc.scalar.activation(
                out=e_sl[:, 32:64], in_=q2[:, 32:64], func=AF.Exp, scale=0.5,
                accum_out=epart[:, 1:2],
            )

            # --- softmax denominators S[t2] and 1/S, replicated at partition
            # offsets {0, 32, 64, 96} for 4-way row-tiled broadcast matmuls.
            ssum4 = psM.tile([98, 1], f32, tag="m")
            for k in range(4):
                nc.tensor.matmul(
                    ssum4[32 * k : 32 * k + 2, :], epart[:, :], ones_sb[:, :],
                    tile_position=(0, 32 * k),
                )
            nc.scalar.copy(sall_sb[:, j : j + 1], ssum4[0:2, :])
            rs4 = packs.tile([98, 1], f32)
            lhsw4 = packs.tile([98, 128], f32)
            for k in range(4):
                nc.vector.reciprocal(
                    rs4[32 * k : 32 * k + 2, :], ssum4[32 * k : 32 * k + 2, :]
                )
                nc.vector.tensor_scalar_mul(
                    lhsw4[32 * k : 32 * k + 2, :],
                    bmask_sb[32 * k : 32 * k + 2, :],
                    rs4[32 * k : 32 * k + 2, :],
                )

            # --- E2 (2, 4096) at 4 partition offsets: transpose packed E then
            # flatten partitions via SBUF->SBUF DMA (HWDGE on the ACT ring)
            etp = psM.tile([64, 128], f32, tag="m")
            nc.tensor.transpose(etp[:, :], e_sl, ident_sb[:, 0:128])
            ets = packs.tile([64, 128], f32)
            nc.scalar.copy(ets[:, :], etp[:, :])
            e2 = e2pool.tile([98, S], f32)
            for k in range(4):
                nc.scalar.dma_start(e2[32 * k : 32 * k + 2, :], ets[:, :])

            # --- broadcast attn across partitions (4-way concurrent row-tiled
            # matmuls) and multiply
            ot = opool.tile([128, S], f32)
            for m in range(8):
                k = m % 4
                bp = psB.tile([128, 512], f32)
                nc.tensor.matmul(
                    bp[:, :],
                    lhsw4[32 * k : 32 * k + 2, :],
                    e2[32 * k : 32 * k + 2, 512 * m : 512 * (m + 1)],
                    tile_position=(32 * k, 0),
                )
                nc.vector.tensor_mul(
                    ot[:, 512 * m : 512 * (m + 1)],
                    xt[:, 512 * m : 512 * (m + 1)],
                    bp[:, :],
                )
            nc.sync.dma_start(
                outx[:, 2 * j : 2 * j + 2, :].rearrange("c t s -> t c s"), ot[:, :]
            )

        nc.sync.dma_start(epk, epk_sb[:, :])
        nc.sync.dma_start(sall, sall_sb[:, :])


def make_consts():
    ident = np.zeros((128, 130), dtype=np.float32)
    ident[:128, :128] = np.eye(128, dtype=np.float32)
    ident[0:64, 128] = 1.0
    ident[64:128, 129] = 1.0
    bmask4 = np.zeros((98, 128), dtype=np.float32)
    for k in range(4):
        bmask4[32 * k + 0, 0:64] = 1.0
        bmask4[32 * k + 1, 64:128] = 1.0
    ones128 = np.ones((128, 1), dtype=np.float32)
    return ident, bmask4, ones128


def core_inputs(x, skin, k):
    """Per-core input dict for core k (host-side shard + pack)."""
    b = k // (NCORES // B)
    tbase = TPC * (k % (NCORES // B))
    xs = np.ascontiguousarray(
        np.asarray(x[b, :, tbase : tbase + TPC], dtype=np.float32).reshape(C, TPC, S)
    )
    sk = np.asarray(skin[b, tbase : tbase + TPC], dtype=np.float32).reshape(TPC, S)
    skh = 2.0 * sk + 2.0
    # skinp[p, j*64 + t2*32 + g] = skh[2j+t2, g*128+p]
    skinp = np.ascontiguousarray(
        skh.reshape(PAIRS, 2, NCHUNK, 128).transpose(3, 0, 1, 2).reshape(128, PAIRS * 64)
    )
    ident, bmask4, ones128 = make_consts()
    return {
        "xs": xs, "skinp": skinp, "ident": ident, "bmask4": bmask4,
        "ones128": ones128,
    }


def assemble(results):
    """Combine per-core outputs into full (output, attn)."""
    output = np.empty((B, C, T, W, H), dtype=np.float32)
    attn = np.empty((B, T, W, H), dtype=np.float32)
    for k in range(NCORES):
        b = k // (NCORES // B)
        tbase = TPC * (k % (NCORES // B))
        r = results[k]
        output[b, :, tbase : tbase + TPC] = r["outx"].reshape(C, TPC, W, H)
        # epk (128, PAIRS*64) -> [p, j, t2, g] -> (t, s)
        e = (
            r["epk"].reshape(128, PAIRS, 2, NCHUNK).transpose(1, 2, 3, 0)
            .reshape(TPC, S)
        )
        s = r["sall"].T.reshape(TPC, 1)  # S[t2, j] -> per-t vector
        attn[b, tbase : tbase + TPC] = (e / s).reshape(TPC, W, H)
    return output, attn


_NC_CACHE = {}


def run(x, skin, trace=False, **spmd_kwargs):
    from concourse.bass_utils import run_bass_kernel_spmd

    if "nc" not in _NC_CACHE:
        _NC_CACHE["nc"] = build_program()
    nc = _NC_CACHE["nc"]
    in_maps = [core_inputs(x, skin, k) for k in range(NCORES)]
    res = run_bass_kernel_spmd(
        nc, in_maps, core_ids=list(range(NCORES)), trace=trace, **spmd_kwargs
    )
    return assemble(res.results), res


def kernel(x, skin):
    out, _ = run(x, skin)
    return out


# revision 23
# speedup vs baseline: 33.7856x; 5.9518x over previous
"""Trainium2 Bass kernel for nn_MixA_Module (channel avg/max pool -> sigmoid ->
softmax over spatial -> broadcast multiply).

Reference (per (b,t) frame, x slice is (C=64, S=4096)):
    avg = sigmoid(mean_c x); mx = sigmoid(max_c x); fusion = avg + mx + skin
    attn = softmax_S(fusion); out = x * attn

Sharding: data-parallel over (B, T). 8 cores; core k handles b = k//2 and a
16-long t range, processed as 8 t-pairs so tiles have 128 partitions.

Per t-pair on chip (natural layout: partition = (t2, c), free = s = w*64+h):
  1. DMA x pair tile (128, 4096), 16 KiB/partition contiguous runs.
  2. PE transpose-matmuls per 128-col chunk with rhs = [I128 | block-ones(2)]:
     psum chunk (128, 130) = [x_chunk^T | per-s channel sums for t2=0,1].
  3. DVE reduce_max over the transposed part -> max over c, packed (128, 64)
     as (s_local, t2*32 + chunk).  ACT tanh on the sums cols gives
     sigmoid(mean) via sigmoid(v) = 0.5*tanh(v/2) + 0.5 (tanh shares the
     "exp_and_others" ACT table set with exp - no table reloads).
  4. fusion exp: E = exp(0.5*(tanh_avg + tanh_max + 2*skin + 2)) on ACT with
     accum_out giving per-partition partial sums; tiny PE matmul + reciprocal
     give 1/S per t2.  E (unnormalized) and S are returned to host, which
     computes attn = E/S (host divide matches jax softmax's fp32 divide).
  5. E is PE-transposed and SBUF->SBUF DMA-flattened to E2 (2, 4096); a PE
     matmul with lhsT = blockmask * (1/S) broadcasts attn = E/S to all 128
     partitions in PSUM; DVE tensor_mul produces out = x * attn.
  6. DMA out tile back in x's layout.
"""

import numpy as np

B, C, T, W, H = 4, 64, 32, 64, 64
S = W * H            # 4096 spatial positions per frame
NCORES = 8
TPC = T * B // NCORES  # 16 t-slices per core (b fixed per core)
PAIRS = TPC // 2       # 8 t-pairs per core
NCHUNK = S // 128      # 32 transpose chunks per pair
GRP = 4                # chunks per psum group
NGRP = NCHUNK // GRP   # 8 groups per pair

# Transpose-mode matmuls require a permutation rhs (HW restriction), so the
# [I | ones] combo matmul and the broadcast matmul run as regular fp32 matmuls.


def build_program(loop_n=None):
    """Build the single-core SPMD Bass program (same program, per-core data).

    loop_n: if set, wrap the whole body in a For_i repeat loop (benchmarking
    only — outputs are identical every iteration)."""
    from contextlib import ExitStack, nullcontext

    import concourse.bacc as bacc
    import concourse.bass as bass
    import concourse.tile as tile
    from concourse import mybir

    f32 = mybir.dt.float32
    AF = mybir.ActivationFunctionType

    nc = bacc.Bacc(
        "TRN2",
        target_bir_lowering=False,
        debug=False,
        enable_asserts=False,
        num_devices=NCORES,
    )

    xs = nc.dram_tensor("xs", [C, TPC, S], f32, kind="ExternalInput").ap()
    skinp = nc.dram_tensor("skinp", [128, PAIRS * 64], f32, kind="ExternalInput").ap()
    ident = nc.dram_tensor("ident", [128, 130], f32, kind="ExternalInput").ap()
    bmask2 = nc.dram_tensor("bmask2", [2, 128], f32, kind="ExternalInput").ap()
    ones128 = nc.dram_tensor("ones128", [128, 1], f32, kind="ExternalInput").ap()
    outx = nc.dram_tensor("outx", [C, TPC, S], f32, kind="ExternalOutput").ap()
    epk = nc.dram_tensor("epk", [128, PAIRS * 64], f32, kind="ExternalOutput").ap()
    sall = nc.dram_tensor("sall", [2, PAIRS], f32, kind="ExternalOutput").ap()

    with tile.TileContext(nc) as tc, ExitStack() as ctx:
        singles = ctx.enter_context(tc.tile_pool(name="singles", bufs=1))
        xpool = ctx.enter_context(tc.tile_pool(name="xin", bufs=3))
        opool = ctx.enter_context(tc.tile_pool(name="oout", bufs=3))
        packs = ctx.enter_context(tc.tile_pool(name="packs", bufs=2))
        e2pool = ctx.enter_context(tc.tile_pool(name="e2", bufs=2))
        psT = ctx.enter_context(tc.tile_pool(name="psT", bufs=2, space="PSUM"))
        psB = ctx.enter_context(tc.tile_pool(name="psB", bufs=3, space="PSUM"))
        psM = ctx.enter_context(tc.tile_pool(name="psM", bufs=1, space="PSUM"))

        ident_sb = singles.tile([128, 130], f32)
        nc.sync.dma_start(ident_sb[:, :], ident)
        bmask_sb = singles.tile([2, 128], f32)
        nc.sync.dma_start(bmask_sb[:, :], bmask2)
        ones_sb = singles.tile([128, 1], f32)
        nc.sync.dma_start(ones_sb[:, :], ones128)
        skin_sb = singles.tile([128, PAIRS * 64], f32)
        nc.sync.dma_start(skin_sb[:, :], skinp)
        epk_sb = singles.tile([128, PAIRS * 64], f32)
        sall_sb = singles.tile([2, PAIRS], f32)

        loop_cm = tc.For_i(0, loop_n, 1) if loop_n else nullcontext()
        with loop_cm:
            _kernel_body(
                nc, tc, mybir, xs, outx, epk, sall, ident_sb, bmask_sb, ones_sb,
                skin_sb, epk_sb, sall_sb, xpool, opool, packs, e2pool, psT, psB,
                psM, f32, AF,
            )

    nc.compile()
    return nc


def _kernel_body(
    nc, tc, mybir, xs, outx, epk, sall, ident_sb, bmask_sb, ones_sb, skin_sb,
    epk_sb, sall_sb, xpool, opool, packs, e2pool, psT, psB, psM, f32, AF,
):
    import concourse.bass as bass

    if True:
        for j in range(PAIRS):
            # --- load x pair: DRAM walk order (t, c, s) -> partitions (t2, c)
            xt = xpool.tile([128, S], f32)
            nc.sync.dma_start(
                xt[:, :], xs[:, 2 * j : 2 * j + 2, :].rearrange("c t s -> t c s")
            )

            mxp = packs.tile([128, 64], f32)   # max over c, (s_local, t2*32+chunk)
            tavg = packs.tile([128, 64], f32)  # tanh(mean/2), same packing

            for g in range(NGRP):
                tg = psT.tile([128, GRP * 256], f32)
                for ci in range(GRP):
                    i = GRP * g + ci
                    # out chunk = [x_chunk^T | channel sums t2=0 | t2=1]
                    nc.tensor.matmul(
                        tg[:, 256 * ci : 256 * ci + 130],
                        xt[:, 128 * i : 128 * (i + 1)],
                        ident_sb[:, :],
                    )
                tg3 = tg[:, :].rearrange("p (ch q) -> p ch q", ch=GRP)
                # max over c: in (p, ch, t2, c) -> out (p, ch, t2)
                in_ap = tg3[:, :, 0:128].rearrange("p ch (t2 c) -> p ch t2 c", t2=2)
                out_ap = (
                    mxp[:, :]
                    .rearrange("p (t2 gg) -> p t2 gg", t2=2)[:, :, GRP * g : GRP * (g + 1)]
                    .rearrange("p t2 ch -> p ch t2")
                )
                nc.vector.reduce_max(out=out_ap, in_=in_ap, axis=mybir.AxisListType.X)
                tavg_ap = (
                    tavg[:, :]
                    .rearrange("p (t2 gg) -> p t2 gg", t2=2)[:, :, GRP * g : GRP * (g + 1)]
                    .rearrange("p t2 ch -> p ch t2")
                )
                # sigmoid(sum/64) = 0.5 + 0.5*tanh(sum/128)
                nc.scalar.activation(
                    out=tavg_ap, in_=tg3[:, :, 128:130], func=AF.Tanh,
                    scale=1.0 / 128.0,
                )

            # --- fusion: E = exp(0.5*(tanh_avg + tanh_max + 2*skin + 2))
            tmx = packs.tile([128, 64], f32)
            nc.scalar.activation(out=tmx[:, :], in_=mxp[:, :], func=AF.Tanh, scale=0.5)
            q = packs.tile([128, 64], f32)
            nc.vector.tensor_add(q[:, :], tavg[:, :], tmx[:, :])
            q2 = packs.tile([128, 64], f32)
            nc.vector.tensor_add(q2[:, :], q[:, :], skin_sb[:, 64 * j : 64 * (j + 1)])

            e_sl = epk_sb[:, 64 * j : 64 * (j + 1)]
            epart = packs.tile([128, 2], f32)
            nc.scalar.activation(
                out=e_sl[:, 0:32], in_=q2[:, 0:32], func=AF.Exp, scale=0.5,
                accum_out=epart[:, 0:1],
            )
            nc.scalar.activation(
                out=e_sl[:, 32:64], in_=q2[:, 32:64], func=AF.Exp, scale=0.5,
                accum_out=epart[:, 1:2],
            )

            # --- softmax denominators S[t2] and 1/S
            ssum = psM.tile([2, 1], f32, tag="m")
            nc.tensor.matmul(ssum[:, :], epart[:, :], ones_sb[:, :])
            nc.scalar.copy(sall_sb[:, j : j + 1], ssum[:, :])
            rs = packs.tile([2, 1], f32)
            nc.vector.reciprocal(rs[:, :], ssum[:, :])
            lhsw = packs.tile([2, 128], f32)
            nc.vector.tensor_scalar_mul(lhsw[:, :], bmask_sb[:, :], rs[:, :])

            # --- E2 (2, 4096): transpose packed E then flatten partitions
            # with one SBUF->SBUF SWDGE DMA
            etp = psM.tile([64, 128], f32, tag="m")
            nc.tensor.transpose(etp[:, :], e_sl, ident_sb[:, 0:128])
            ets = packs.tile([64, 128], f32)
            nc.scalar.copy(ets[:, :], etp[:, :])
            e2 = e2pool.tile([2, S], f32)
            nc.gpsimd.dma_start(e2[:, :], ets[:, :])

            # --- broadcast attn across partitions; 4 column-tiled matmuls per
            # chunk run concurrently in the PE array (M=32 each), then multiply
            ot = opool.tile([128, S], f32)
            for m in range(8):
                bp = psB.tile([128, 512], f32)
                for k in range(4):
                    nc.tensor.matmul(
                        bp[32 * k : 32 * k + 32, :],
                        lhsw[:, 32 * k : 32 * k + 32],
                        e2[:, 512 * m : 512 * (m + 1)],
                        tile_position=(0, 32 * k),
                    )
                nc.vector.tensor_mul(
                    ot[:, 512 * m : 512 * (m + 1)],
                    xt[:, 512 * m : 512 * (m + 1)],
                    bp[:, :],
                )
            nc.sync.dma_start(
                outx[:, 2 * j : 2 * j + 2, :].rearrange("c t s -> t c s"), ot[:, :]
            )

        nc.sync.dma_start(epk, epk_sb[:, :])
        nc.sync.dma_start(sall, sall_sb[:, :])


def make_consts():
    ident = np.zeros((128, 130), dtype=np.float32)
    ident[:128, :128] = np.eye(128, dtype=np.float32)
    ident[0:64, 128] = 1.0
    ident[64:128, 129] = 1.0
    bmask2 = np.zeros((2, 128), dtype=np.float32)
    bmask2[0, 0:64] = 1.0
    bmask2[1, 64:128] = 1.0
    ones128 = np.ones((128, 1), dtype=np.float32)
    return ident, bmask2, ones128


def core_inputs(x, skin, k):
    """Per-core input dict for core k (host-side shard + pack)."""
    b = k // (NCORES // B)
    tbase = TPC * (k % (NCORES // B))
    xs = np.ascontiguousarray(
        np.asarray(x[b, :, tbase : tbase + TPC], dtype=np.float32).reshape(C, TPC, S)
    )
    sk = np.asarray(skin[b, tbase : tbase + TPC], dtype=np.float32).reshape(TPC, S)
    skh = 2.0 * sk + 2.0
    # skinp[p, j*64 + t2*32 + g] = skh[2j+t2, g*128+p]
    skinp = np.ascontiguousarray(
        skh.reshape(PAIRS, 2, NCHUNK, 128).transpose(3, 0, 1, 2).reshape(128, PAIRS * 64)
    )
    ident, bmask2, ones128 = make_consts()
    return {
        "xs": xs, "skinp": skinp, "ident": ident, "bmask2": bmask2,
        "ones128": ones128,
    }


def assemble(results):
    """Combine per-core outputs into full (output, attn)."""
    output = np.empty((B, C, T, W, H), dtype=np.float32)
    attn = np.empty((B, T, W, H), dtype=np.float32)
    for k in range(NCORES):
        b = k // (NCORES // B)
        tbase = TPC * (k % (NCORES // B))
        r = results[k]
        output[b, :, tbase : tbase + TPC] = r["outx"].reshape(C, TPC, W, H)
        # epk (128, PAIRS*64) -> [p, j, t2, g] -> (t, s)
        e = (
            r["epk"].reshape(128, PAIRS, 2, NCHUNK).transpose(1, 2, 3, 0)
            .reshape(TPC, S)
        )
        s = r["sall"].T.reshape(TPC, 1)  # S[t2, j] -> per-t vector
        attn[b, tbase : tbase + TPC] = (e / s).reshape(TPC, W, H)
    return output, attn


_NC_CACHE = {}


def run(x, skin, trace=False, **spmd_kwargs):
    from concourse.bass_utils import run_bass_kernel_spmd

    if "nc" not in _NC_CACHE:
        _NC_CACHE["nc"] = build_program()
    nc = _NC_CACHE["nc"]
    in_maps = [core_inputs(x, skin, k) for k in range(NCORES)]
    res = run_bass_kernel_spmd(
        nc, in_maps, core_ids=list(range(NCORES)), trace=trace, **spmd_kwargs
    )
    return assemble(res.results), res


def kernel(x, skin):
    out, _ = run(x, skin)
    return out
